# revision 2
# baseline (speedup 1.0000x reference)
"""Trainium2 Bass kernel for nn_BERTEmbedding_65274912964883.

out[b, l, :] = token_table[seq[b, l]]
             + mean_{g in genres(seq[b, l])} genre_table[g]
             + pos_table[l]

Strategy (8 NeuronCores, SPMD, no collectives):
  - Data-parallel over batch: 256 sequences -> 32 per core (6400 tokens/core).
  - One combined table [VOCAB, 144] f32 replicated per core:
    cols 0..127 token embedding, 128..135 genre ids (f32), 136 count (f32).
  - Per 128-token subtile (token t on partition t % 128): ONE indirect-DMA
    gather of 576B rows (HW DGE consumes one index per partition per DMA).
  - genre mean = (weighted one-hot histogram over 21 genres) @ genre_table:
    histogram via one big is_equal + mask-multiply + reduce on DVE (batched
    over KSUB subtiles), PE transposes the histogram and contracts with the
    SBUF-resident genre table.
  - positional rows from 25 pre-rotated SBUF-resident [128, D] tiles
    (128*i mod 200 has period 25).
  - Device writes output partition-major [128, N/128, D]; host un-permutes.
"""

import numpy as np

import concourse.bacc as bacc
import concourse.mybir as mybir
import concourse.tile as tile
from concourse.bass import IndirectOffsetOnAxis
from concourse.bass_utils import run_bass_kernel_spmd

VOCAB = 100000
D = 128
G = 21          # genre ids are in [0, 20]
MAXG = 8
CW = 144        # combined-table row: 128 emb + 8 gid + 1 cnt + 7 pad
B, L = 256, 200
NCORES = 8
BC = B // NCORES          # sequences per core
N = BC * L                # tokens per core (6400)
SUB = 128                 # tokens per subtile (partition dim)
KSUB = 10                 # subtiles per macro tile
MACRO = SUB * KSUB        # 1280 tokens
NMACRO = N // MACRO       # 5
NSUB = N // SUB           # 50
NROT = 25                 # distinct values of (128*i) % 200

F32 = mybir.dt.float32
I32 = mybir.dt.int32


def emit_core_kernel(tc, seq, ctab, gtab, pos, giota, iota8, ident, out):
    """Emit the per-core kernel into TileContext `tc`.

    seq   : DRAM [128, NSUB] int32, seq[p, i] = token id of token i*128+p
    ctab  : DRAM [VOCAB, CW] f32 combined table
    gtab  : DRAM [G, D] f32
    pos   : DRAM [L, D] f32
    giota : DRAM [128, G] f32, each row = 0..G-1
    iota8 : DRAM [128, MAXG] f32, each row = 0..MAXG-1
    ident : DRAM [128, 128] f32 identity
    out   : DRAM [128, NSUB, D] f32, out[p, i, :] = embedding of token i*128+p
    """
    nc = tc.nc
    add = mybir.AluOpType.add
    mult = mybir.AluOpType.mult

    with (
        tc.tile_pool(name="const", bufs=1) as cpool,
        tc.tile_pool(name="work", bufs=2) as wpool,
        tc.tile_pool(name="psum", bufs=2, space="PSUM") as ppool,
    ):
        # --- constants / one-time loads ---
        gtab_sb = cpool.tile([G, D], F32)
        nc.sync.dma_start(out=gtab_sb[:], in_=gtab)
        giota_sb = cpool.tile([128, G], F32)
        nc.sync.dma_start(out=giota_sb[:], in_=giota)
        iota8_sb = cpool.tile([128, MAXG], F32)
        nc.sync.dma_start(out=iota8_sb[:], in_=iota8)
        ident_sb = cpool.tile([128, 128], F32)
        nc.sync.dma_start(out=ident_sb[:], in_=ident)

        seq_sb = cpool.tile([128, NSUB], I32)
        nc.sync.dma_start(out=seq_sb[:], in_=seq)

        # 25 pre-rotated positional tiles: posrot[:, r*D:(r+1)*D][p, :] =
        # pos[(128*r + p) % 200, :]
        posrot_sb = cpool.tile([128, NROT * D], F32)
        for r in range(NROT):
            s = (r * SUB) % L
            n1 = min(SUB, L - s)
            nc.sync.dma_start(
                out=posrot_sb[0:n1, r * D:(r + 1) * D], in_=pos[s:s + n1, :]
            )
            if n1 < SUB:
                nc.sync.dma_start(
                    out=posrot_sb[n1:SUB, r * D:(r + 1) * D], in_=pos[0:SUB - n1, :]
                )

        # --- main loop over macro tiles of 1280 tokens ---
        for m in range(NMACRO):
            # gather combined rows, one indirect DMA per 128-token subtile
            cg_sb = wpool.tile([128, KSUB * CW], F32, tag="cg")
            for j in range(KSUB):
                nc.gpsimd.indirect_dma_start(
                    out=cg_sb[:, j * CW:(j + 1) * CW],
                    out_offset=None,
                    in_=ctab,
                    in_offset=IndirectOffsetOnAxis(
                        ap=seq_sb[:, m * KSUB + j:m * KSUB + j + 1], axis=0
                    ),
                )
            cg3 = cg_sb[:].rearrange("p (j c) -> p j c", c=CW)
            gid = cg3[:, :, D:D + MAXG]                # [128, KSUB, MAXG]
            cnt = cg3[:, :, D + MAXG:D + MAXG + 1]     # [128, KSUB, 1]

            # rec[p, j] = 1 / count
            rec_sb = wpool.tile([128, KSUB], F32, tag="rec")
            nc.vector.reciprocal(rec_sb[:], cg3[:, :, D + MAXG])

            # mask[p, j, s] = (s < count[p, j])
            mask_sb = wpool.tile([128, KSUB * MAXG], F32, tag="mask")
            m3 = mask_sb[:].rearrange("p (j s) -> p j s", s=MAXG)
            nc.vector.tensor_tensor(
                out=m3,
                in0=iota8_sb[:].unsqueeze(1).broadcast_to([128, KSUB, MAXG]),
                in1=cnt.broadcast_to([128, KSUB, MAXG]),
                op=mybir.AluOpType.is_lt,
            )
            # wmask = mask / count
            wmask_sb = wpool.tile([128, KSUB * MAXG], F32, tag="wmask")
            w3 = wmask_sb[:].rearrange("p (j s) -> p j s", s=MAXG)
            nc.vector.tensor_tensor(
                out=w3,
                in0=m3,
                in1=rec_sb[:].unsqueeze(2).broadcast_to([128, KSUB, MAXG]),
                op=mult,
            )

            # eq[p, j, s, g] = (gid[p, j, s] == g)
            eq_sb = wpool.tile([128, KSUB * MAXG * G], F32, tag="eq")
            e4 = eq_sb[:].rearrange("p (j s g) -> p j s g", s=MAXG, g=G)
            nc.vector.tensor_tensor(
                out=e4,
                in0=gid.unsqueeze(3).broadcast_to([128, KSUB, MAXG, G]),
                in1=giota_sb[:].unsqueeze(1).unsqueeze(2).broadcast_to(
                    [128, KSUB, MAXG, G]
                ),
                op=mybir.AluOpType.is_equal,
            )
            # eqw = eq * wmask
            eqw_sb = wpool.tile([128, KSUB * MAXG * G], F32, tag="eqw")
            q4 = eqw_sb[:].rearrange("p (j s g) -> p j s g", s=MAXG, g=G)
            nc.vector.tensor_tensor(
                out=q4,
                in0=e4,
                in1=w3.unsqueeze(3).broadcast_to([128, KSUB, MAXG, G]),
                op=mult,
            )

            # hist[p, j, g] = sum_s eqw[p, j, s, g]   (normalized histogram)
            hist_sb = wpool.tile([128, KSUB * G], F32, tag="hist")
            nc.vector.reduce_sum(
                out=hist_sb[:].rearrange("p (j g) -> p j g", g=G),
                in_=eqw_sb[:].rearrange("p (j s g) -> p j g s", s=MAXG, g=G),
                axis=mybir.AxisListType.X,
            )

            out_sb = wpool.tile([128, KSUB * D], F32, tag="outsb")
            for j in range(KSUB):
                # histT[g, p] via PE transpose
                histT_ps = ppool.tile([G, 128], F32, tag="histT_ps")
                nc.tensor.transpose(
                    out=histT_ps[:],
                    in_=hist_sb[:, j * G:(j + 1) * G],
                    identity=ident_sb[:],
                )
                histT_sb = wpool.tile([G, 128], F32, tag="histT_sb")
                nc.vector.tensor_copy(out=histT_sb[:], in_=histT_ps[:])

                # genre_mean[p, d] = sum_g histT[g, p] * gtab[g, d]
                gm_ps = ppool.tile([128, D], F32, tag="gm_ps")
                nc.tensor.matmul(
                    out=gm_ps[:], lhsT=histT_sb[:], rhs=gtab_sb[:],
                    start=True, stop=True,
                )

                r = (m * KSUB + j) % NROT
                oslice = out_sb[:, j * D:(j + 1) * D]
                nc.vector.tensor_tensor(
                    out=oslice, in0=cg3[:, j, 0:D], in1=gm_ps[:],
                    op=add,
                )
                nc.vector.tensor_tensor(
                    out=oslice, in0=oslice, in1=posrot_sb[:, r * D:(r + 1) * D],
                    op=add,
                )

            # store macro tile: out[p, m*KSUB + j, :] (contiguous per partition)
            nc.sync.dma_start(
                out=out[:, m * KSUB:(m + 1) * KSUB, :],
                in_=out_sb[:].rearrange("p (j d) -> p j d", d=D),
            )


def build_nc():
    nc = bacc.Bacc("TRN2", target_bir_lowering=False, debug=False)
    seq = nc.dram_tensor("seq", [128, NSUB], I32, kind="ExternalInput").ap()
    ctab = nc.dram_tensor("ctab", [VOCAB, CW], F32, kind="ExternalInput").ap()
    gtab = nc.dram_tensor("gtab", [G, D], F32, kind="ExternalInput").ap()
    pos = nc.dram_tensor("pos", [L, D], F32, kind="ExternalInput").ap()
    giota = nc.dram_tensor("giota", [128, G], F32, kind="ExternalInput").ap()
    iota8 = nc.dram_tensor("iota8", [128, MAXG], F32, kind="ExternalInput").ap()
    ident = nc.dram_tensor("ident", [128, 128], F32, kind="ExternalInput").ap()
    out = nc.dram_tensor("out", [128, NSUB, D], F32, kind="ExternalOutput").ap()

    with tile.TileContext(nc) as tc:
        emit_core_kernel(tc, seq, ctab, gtab, pos, giota, iota8, ident, out)
    nc.compile()
    return nc


_NC_CACHE = None


def _get_nc():
    global _NC_CACHE
    if _NC_CACHE is None:
        _NC_CACHE = build_nc()
    return _NC_CACHE


def make_ctab(token_table, token_genre_ids, genre_counts):
    ctab = np.zeros((VOCAB, CW), dtype=np.float32)
    ctab[:, 0:D] = np.asarray(token_table, dtype=np.float32)
    ctab[:, D:D + MAXG] = np.asarray(token_genre_ids, dtype=np.float32)
    ctab[:, D + MAXG] = np.asarray(genre_counts, dtype=np.float32)
    return ctab


def prep_host_inputs(sequence, token_table, genre_table, pos_table,
                     token_genre_ids, genre_counts):
    """Host-side sharding / layout prep. Returns in_maps for the 8 cores."""
    seq = np.ascontiguousarray(np.asarray(sequence).astype(np.int32)).reshape(B, L)
    ctab = make_ctab(token_table, token_genre_ids, genre_counts)
    gtab = np.ascontiguousarray(np.asarray(genre_table, dtype=np.float32))
    pos = np.ascontiguousarray(np.asarray(pos_table, dtype=np.float32))

    giota = np.broadcast_to(np.arange(G, dtype=np.float32), (128, G)).copy()
    iota8 = np.broadcast_to(np.arange(MAXG, dtype=np.float32), (128, MAXG)).copy()
    ident = np.eye(128, dtype=np.float32)

    in_maps = []
    for c in range(NCORES):
        seq_core = seq[c * BC:(c + 1) * BC].reshape(N)
        # device layout: seq_dev[p, i] = seq_core[i*128 + p]
        seq_dev = np.ascontiguousarray(seq_core.reshape(NSUB, 128).T)
        in_maps.append({
            "seq": seq_dev,
            "ctab": ctab,
            "gtab": gtab,
            "pos": pos,
            "giota": giota,
            "iota8": iota8,
            "ident": ident,
        })
    return in_maps


def postprocess(results):
    """Un-permute per-core outputs and concatenate to [B, L, D]."""
    outs = []
    for c in range(NCORES):
        o = results[c]["out"]  # [128, NSUB, D]
        outs.append(np.ascontiguousarray(o.transpose(1, 0, 2)).reshape(BC, L, D))
    return np.concatenate(outs, axis=0)


def kernel(sequence, token_table, genre_table, pos_table, token_genre_ids,
           genre_counts):
    nc = _get_nc()
    in_maps = prep_host_inputs(sequence, token_table, genre_table, pos_table,
                               token_genre_ids, genre_counts)
    res = run_bass_kernel_spmd(nc, in_maps, core_ids=list(range(NCORES)))
    return postprocess(res.results)


# revision 9
# speedup vs baseline: 1.3991x; 1.3991x over previous
"""Trainium2 Bass kernel for nn_BERTEmbedding_65274912964883.

out[b, l, :] = token_table[seq[b, l]]
             + mean_{g in genres(seq[b, l])} genre_table[g]
             + pos_table[l]

Strategy (8 NeuronCores, SPMD, no collectives):
  - Data-parallel over batch: 256 sequences -> 32 per core (6400 tokens/core).
  - One combined bf16 table [VOCAB, 144] replicated per core:
    cols 0..127 token embedding, 128..135 genre ids, 136 count.
  - Per 128-token subtile (token t on partition t % 128): ONE indirect-DMA
    gather of 288B rows (HW DGE consumes one index per partition per DMA,
    so SWDGE descriptor emission ~= 8.6ns/token is the floor; everything
    else is kept beneath it).
  - genre mean = (one-hot histogram over 21 genres) @ genre_table:
    padded genre slots are remapped out of range (gid + 32*(1-mask)) so no
    masked multiply is needed; histogram normalization (x 1/count) is one
    small DVE op; PE transposes the histogram per macro-tile (2 transposes
    for 10 subtiles) and contracts with the SBUF-resident bf16 genre table.
  - token + genre + positional adds run group-batched ([128, 512] PSUM banks).
  - positional rows come from a host-prebuilt rotated table (28 rotations of
    pos, bf16) so no wrap handling and a single startup DMA.
  - Device writes output partition-major [128, N/128, D] f32; host un-permutes.
"""

import numpy as np
import ml_dtypes

import concourse.bacc as bacc
import concourse.mybir as mybir
import concourse.tile as tile
from concourse.bass import IndirectOffsetOnAxis
from concourse.bass_utils import run_bass_kernel_spmd

VOCAB = 100000
D = 128
G = 21          # genre ids are in [0, 20]
MAXG = 8
CW = 144        # combined-table row: 128 emb + 8 gid + 1 cnt + 7 pad
B, L = 256, 200
NCORES = 8
BC = B // NCORES          # sequences per core
N = BC * L                # tokens per core (6400)
SUB = 128                 # tokens per subtile (partition dim)
KSUB = 10                 # subtiles per macro tile
MACRO = SUB * KSUB        # 1280 tokens
NMACRO = N // MACRO       # 5
NSUB = N // SUB           # 50
NROT = 25                 # distinct values of (128*i) % 200
NROTX = 28                # extended with 3 duplicates so groups never wrap
GROUPS = [(0, 4), (4, 4), (8, 2)]   # (start subtile, size) per PSUM bank group

F32 = mybir.dt.float32
BF16 = mybir.dt.bfloat16
I32 = mybir.dt.int32


def emit_core_kernel(tc, seq, ctab, gtab, posrot, giota, iota8, ident, out):
    """Emit the per-core kernel into TileContext `tc`.

    seq    : DRAM [128, NSUB] int32, seq[p, i] = token id of token i*128+p
    ctab   : DRAM [VOCAB, CW] bf16 combined table
    gtab   : DRAM [G, D] bf16
    posrot : DRAM [128, NROTX*D] bf16, posrot[p, r*D+d] = pos[(128r+p)%200, d]
    giota  : DRAM [128, G] bf16, each row = 0..G-1
    iota8  : DRAM [128, MAXG] bf16, each row = 0..MAXG-1
    ident  : DRAM [128, 128] bf16 identity
    out    : DRAM [128, NSUB, D] f32, out[p, i, :] = embedding of token i*128+p
    """
    nc = tc.nc
    add = mybir.AluOpType.add
    mult = mybir.AluOpType.mult

    with (
        tc.tile_pool(name="const", bufs=1) as cpool,
        tc.tile_pool(name="work", bufs=2) as wpool,
        tc.tile_pool(name="psum", bufs=2, space="PSUM") as ppool,
    ):
        # --- one-time loads; seq first (gathers depend only on it) ---
        seq_sb = cpool.tile([128, NSUB], I32)
        nc.sync.dma_start(out=seq_sb[:], in_=seq)
        # genre table replicated at partitions 0/32/64/96 (PE quadrant rhs)
        gtab_sb = cpool.tile([128, D], BF16)
        nc.sync.dma_start(out=gtab_sb[:], in_=gtab)
        giota_sb = cpool.tile([128, G], BF16)
        nc.sync.dma_start(out=giota_sb[:], in_=giota)
        iota8_sb = cpool.tile([128, MAXG], BF16)
        nc.sync.dma_start(out=iota8_sb[:], in_=iota8)
        ident_sb = cpool.tile([128, 128], BF16)
        nc.sync.dma_start(out=ident_sb[:], in_=ident)
        posrot_sb = cpool.tile([128, NROTX * D], BF16)
        nc.sync.dma_start(out=posrot_sb[:], in_=posrot)

        # --- main loop over macro tiles of 1280 tokens ---
        for m in range(NMACRO):
            # gather combined rows, one indirect DMA per 128-token subtile
            cg_sb = wpool.tile([128, KSUB * CW], BF16, tag="cg", bufs=3)
            for j in range(KSUB):
                nc.gpsimd.indirect_dma_start(
                    out=cg_sb[:, j * CW:(j + 1) * CW],
                    out_offset=None,
                    in_=ctab,
                    in_offset=IndirectOffsetOnAxis(
                        ap=seq_sb[:, m * KSUB + j:m * KSUB + j + 1], axis=0
                    ),
                )
            cg3 = cg_sb[:].rearrange("p (j c) -> p j c", c=CW)
            gid = cg3[:, :, D:D + MAXG]                # [128, KSUB, MAXG] bf16
            cnt = cg3[:, :, D + MAXG:D + MAXG + 1]     # [128, KSUB, 1] bf16

            # rec[p, j] = 1 / count   (f32)
            rec_sb = wpool.tile([128, KSUB], F32, tag="rec")
            nc.vector.reciprocal(rec_sb[:], cg3[:, :, D + MAXG])

            # mask[p, j, s] = (s < count[p, j])
            mask_sb = wpool.tile([128, KSUB * MAXG], BF16, tag="mask")
            m3 = mask_sb[:].rearrange("p (j s) -> p j s", s=MAXG)
            nc.vector.tensor_tensor(
                out=m3,
                in0=iota8_sb[:].unsqueeze(1).broadcast_to([128, KSUB, MAXG]),
                in1=cnt.broadcast_to([128, KSUB, MAXG]),
                op=mybir.AluOpType.is_lt,
            )
            # shift[p, j, s] = 32 * (1 - mask); gidm = gid + shift
            # (padded slots land at >= 32 and never match any genre column)
            shift_sb = wpool.tile([128, KSUB * MAXG], BF16, tag="shift")
            nc.vector.tensor_scalar(
                out=shift_sb[:], in0=mask_sb[:],
                scalar1=-32.0, scalar2=32.0,
                op0=mult, op1=add,
            )
            gidm_sb = wpool.tile([128, KSUB * MAXG], BF16, tag="gidm")
            nc.vector.tensor_tensor(
                out=gidm_sb[:].rearrange("p (j s) -> p j s", s=MAXG),
                in0=gid,
                in1=shift_sb[:].rearrange("p (j s) -> p j s", s=MAXG),
                op=add,
            )

            # eq[p, j, s, g] = (gidm[p, j, s] == g)
            eq_sb = wpool.tile([128, KSUB * MAXG * G], BF16, tag="eq")
            e4 = eq_sb[:].rearrange("p (j s g) -> p j s g", s=MAXG, g=G)
            nc.vector.tensor_tensor(
                out=e4,
                in0=gidm_sb[:].rearrange("p (j s) -> p j s", s=MAXG)
                    .unsqueeze(3).broadcast_to([128, KSUB, MAXG, G]),
                in1=giota_sb[:].unsqueeze(1).unsqueeze(2).broadcast_to(
                    [128, KSUB, MAXG, G]
                ),
                op=mybir.AluOpType.is_equal,
            )

            # hist_raw[p, j, g] = sum_s eq[p, j, s, g]
            hist_sb = wpool.tile([128, KSUB * G], F32, tag="hist")
            nc.vector.reduce_sum(
                out=hist_sb[:].rearrange("p (j g) -> p j g", g=G),
                in_=eq_sb[:].rearrange("p (j s g) -> p j g s", s=MAXG, g=G),
                axis=mybir.AxisListType.X,
            )
            # hist_norm = hist_raw / count   (bf16, packed [128, KSUB*G])
            histn_sb = wpool.tile([128, KSUB * G], BF16, tag="histn")
            nc.vector.tensor_tensor(
                out=histn_sb[:].rearrange("p (j g) -> p j g", g=G),
                in0=hist_sb[:].rearrange("p (j g) -> p j g", g=G),
                in1=rec_sb[:].unsqueeze(2).broadcast_to([128, KSUB, G]),
                op=mult,
            )

            # per-subtile PE transpose of the histogram (base partition 0);
            # PSUM -> SBUF copies ride the otherwise-idle Scalar engine
            histT = []
            for j in range(KSUB):
                hT_ps = ppool.tile([G, 128], BF16, tag="hT_ps", bufs=3)
                nc.tensor.transpose(
                    out=hT_ps[:],
                    in_=histn_sb[:, j * G:(j + 1) * G],
                    identity=ident_sb[:],
                )
                hT_sb = wpool.tile([G, 128], BF16, tag="hT_sb", bufs=3)
                nc.scalar.copy(out=hT_sb[:], in_=hT_ps[:])
                histT.append(hT_sb)

            out_sb = wpool.tile([128, KSUB * D], F32, tag="outsb")
            for j0, ng in GROUPS:
                gm_ps = ppool.tile([128, ng * D], F32, tag="gm_ps")
                for k in range(ng):
                    j = j0 + k
                    nc.tensor.matmul(
                        out=gm_ps[:, k * D:(k + 1) * D],
                        lhsT=histT[j][:],
                        rhs=gtab_sb[0:G, :],
                        start=True, stop=True,
                    )
                # out = tok + genre_mean + pos   (two group-batched adds)
                oslice = out_sb[:, j0 * D:(j0 + ng) * D]
                nc.vector.tensor_tensor(
                    out=oslice,
                    in0=cg3[:, j0:j0 + ng, 0:D],
                    in1=gm_ps[:],
                    op=add,
                )
                r0 = (m * KSUB + j0) % NROT
                nc.vector.tensor_tensor(
                    out=oslice,
                    in0=oslice,
                    in1=posrot_sb[:, r0 * D:(r0 + ng) * D],
                    op=add,
                )

            # store macro tile: out[p, m*KSUB + j, :] (contiguous per partition)
            nc.sync.dma_start(
                out=out[:, m * KSUB:(m + 1) * KSUB, :],
                in_=out_sb[:].rearrange("p (j d) -> p j d", d=D),
            )


def build_nc():
    nc = bacc.Bacc("TRN2", target_bir_lowering=False, debug=False)
    seq = nc.dram_tensor("seq", [128, NSUB], I32, kind="ExternalInput").ap()
    ctab = nc.dram_tensor("ctab", [VOCAB, CW], BF16, kind="ExternalInput").ap()
    gtab = nc.dram_tensor("gtab", [128, D], BF16, kind="ExternalInput").ap()
    posrot = nc.dram_tensor(
        "posrot", [128, NROTX * D], BF16, kind="ExternalInput").ap()
    giota = nc.dram_tensor("giota", [128, G], BF16, kind="ExternalInput").ap()
    iota8 = nc.dram_tensor("iota8", [128, MAXG], BF16, kind="ExternalInput").ap()
    ident = nc.dram_tensor("ident", [128, 128], BF16, kind="ExternalInput").ap()
    out = nc.dram_tensor("out", [128, NSUB, D], F32, kind="ExternalOutput").ap()

    with tile.TileContext(nc) as tc:
        emit_core_kernel(tc, seq, ctab, gtab, posrot, giota, iota8, ident, out)
    nc.compile()
    return nc


_NC_CACHE = None


def _get_nc():
    global _NC_CACHE
    if _NC_CACHE is None:
        _NC_CACHE = build_nc()
    return _NC_CACHE


def make_ctab(token_table, token_genre_ids, genre_counts):
    ctab = np.zeros((VOCAB, CW), dtype=ml_dtypes.bfloat16)
    ctab[:, 0:D] = np.asarray(token_table, dtype=np.float32).astype(
        ml_dtypes.bfloat16)
    ctab[:, D:D + MAXG] = np.asarray(
        token_genre_ids, dtype=np.float32).astype(ml_dtypes.bfloat16)
    ctab[:, D + MAXG] = np.asarray(
        genre_counts, dtype=np.float32).astype(ml_dtypes.bfloat16)
    return ctab


def make_posrot(pos_table):
    pos = np.asarray(pos_table, dtype=np.float32)
    pr = np.zeros((128, NROTX * D), dtype=np.float32)
    p = np.arange(128)
    for r in range(NROTX):
        pr[:, r * D:(r + 1) * D] = pos[(128 * r + p) % L, :]
    return pr.astype(ml_dtypes.bfloat16)


def prep_host_inputs(sequence, token_table, genre_table, pos_table,
                     token_genre_ids, genre_counts):
    """Host-side sharding / layout prep. Returns in_maps for the 8 cores."""
    seq = np.ascontiguousarray(np.asarray(sequence).astype(np.int32)).reshape(B, L)
    ctab = make_ctab(token_table, token_genre_ids, genre_counts)
    g32 = np.asarray(genre_table, dtype=np.float32)
    gtab = np.zeros((128, D), dtype=np.float32)
    for k in range(4):
        gtab[32 * k:32 * k + G] = g32
    gtab = gtab.astype(ml_dtypes.bfloat16)
    posrot = make_posrot(pos_table)

    giota = np.broadcast_to(
        np.arange(G, dtype=np.float32), (128, G)).astype(ml_dtypes.bfloat16)
    iota8 = np.broadcast_to(
        np.arange(MAXG, dtype=np.float32), (128, MAXG)).astype(ml_dtypes.bfloat16)
    ident = np.eye(128, dtype=np.float32).astype(ml_dtypes.bfloat16)

    in_maps = []
    for c in range(NCORES):
        seq_core = seq[c * BC:(c + 1) * BC].reshape(N)
        # device layout: seq_dev[p, i] = seq_core[i*128 + p]
        seq_dev = np.ascontiguousarray(seq_core.reshape(NSUB, 128).T)
        in_maps.append({
            "seq": seq_dev,
            "ctab": ctab,
            "gtab": gtab,
            "posrot": posrot,
            "giota": giota,
            "iota8": iota8,
            "ident": ident,
        })
    return in_maps


def postprocess(results):
    """Un-permute per-core outputs and concatenate to [B, L, D]."""
    outs = []
    for c in range(NCORES):
        o = results[c]["out"]  # [128, NSUB, D]
        outs.append(np.ascontiguousarray(o.transpose(1, 0, 2)).reshape(BC, L, D))
    return np.concatenate(outs, axis=0)


def kernel(sequence, token_table, genre_table, pos_table, token_genre_ids,
           genre_counts):
    nc = _get_nc()
    in_maps = prep_host_inputs(sequence, token_table, genre_table, pos_table,
                               token_genre_ids, genre_counts)
    res = run_bass_kernel_spmd(nc, in_maps, core_ids=list(range(NCORES)))
    return postprocess(res.results)
